# revision 11
# baseline (speedup 1.0000x reference)
"""nn_AttentionOut_63711544869147 — causal multi-head attention + output projection,
distributed over 8 Trainium2 NeuronCores.

Module: out = softmax(causal(Q K^T / sqrt(d))) V @ W_O + b_O, returned with the
(unchanged) residual: reference returns the tuple (residual, out).

Sharding (8 cores = 2 batches x 4 head-groups of 4 heads, SPMD single program):
  each core computes full causal attention for its batch over its 4 heads and
  a partial projection  sum_{h in group} z_h @ W_O[h]  ->  pout [2048, 1024].
  The host sums the 4 head-group partials per batch (the "all-reduce" of the
  row-sharded W_O product), adds b_O, and passes the residual through.

Device dataflow per (head, 512-wide q strip), exact causal tiling:
  scores_T[kv,q] = K_h^T_tile.T @ Q_h^T_strip      (PE bf16; kv-tile pairs into
                                                    a 2-bank PSUM tile)
  expP = exp(scores_T * 1/8)                        (ACT; scale folds 1/sqrt(64);
                                                    one exp per kv-tile pair;
                                                    diagonal tiles pack their
                                                    partial ranges into 2 tiles)
  triangular mask on diagonal blocks                (DVE mul by 0/1 matrix)
  z_ext[65,q] += V_ext_tile.T @ expP                (PE accum; V_ext = [V | 1]
                                                    so row 64 = softmax denom)
  zn = z[0:64] * (1/z[64]) (DVE copy + approx-recip + DRAM-bounce broadcast
                            + mul; DVE crossbar allows the partition-offset
                            write for odd heads)
  pout strip = zn_T @ W_O_group                     (PE, 2x128-deep contraction)

PE emission is software-pipelined: each PV pair is deferred until after the
NEXT pair's QK+exp is emitted (one-pair lookahead keeps the in-order PE queue
from stalling on ACT), and the previous strip's projection matmuls are
interleaved at head boundaries as PE filler during ACT-bound stretches. This
keeps the PE p-state ramp warm (2.4 GHz needs >3us continuous busy). Input
DMAs are split and emitted in need order (the cost model serializes all DMA
transfers on one resource).
"""

import numpy as np

import concourse.bass as bass
import concourse.bacc as bacc
import concourse.tile as tile
from concourse import mybir
from concourse.bass_utils import run_bass_kernel_spmd

F32 = mybir.dt.float32
BF16 = mybir.dt.bfloat16

N_CORES = 8
N_HEADS = 16
H = 4          # heads per core
S = 2048
D = 64
P = 128
D_MODEL = 1024
NSTRIP = 4     # q strips of 512
QW = 512       # strip width

# feature flags (conservative=False values use only baseline-proven constructs)
PACK_DIAG = True        # pack the 4 diagonal tiles into 2 sc tiles / 2 exps
USE_GPS_MASK = True     # tri masks on gpsimd vs DVE
USE_GPS_BCAST = True    # gpsimd partition_broadcast vs DRAM-bounce broadcast
USE_BF16_OUT = False    # pout in bf16 (halves output DMA) vs f32

_PROGRAM = None
_PROGRAM_KEY = None
LAST_RESULTS = None


def build_program():
    MMDT = BF16
    ODT = BF16 if USE_BF16_OUT else F32
    nc = bacc.Bacc(target_bir_lowering=False)

    qT = nc.dram_tensor("qT", [H, D, S], MMDT, kind="ExternalInput")
    kT = nc.dram_tensor("kT", [H, D, S], MMDT, kind="ExternalInput")
    # v prearranged by host: [head, partition(kv%128), kvtile(16), d(64)]
    v = nc.dram_tensor("v", [H, P, 16 * D], MMDT, kind="ExternalInput")
    wo = nc.dram_tensor("wo", [2 * P, D_MODEL], MMDT, kind="ExternalInput")
    tri = nc.dram_tensor("tri", [P, P], MMDT, kind="ExternalInput")
    pout = nc.dram_tensor("pout", [S, D_MODEL], ODT, kind="ExternalOutput")

    MASKENG = nc.gpsimd if USE_GPS_MASK else nc.vector

    with tile.TileContext(nc) as tc:
        with (
            tc.tile_pool(name="persist", bufs=1) as persist,
            tc.tile_pool(name="expp", bufs=4) as expp,
            tc.tile_pool(name="rcpp", bufs=2) as rcpp,
            tc.tile_pool(name="outp", bufs=4) as outp,
            tc.tile_pool(name="znp", bufs=2) as znp,
            tc.tile_pool(name="scps", bufs=2, space="PSUM") as scps,
            tc.tile_pool(name="zps", bufs=2, space="PSUM") as zps,
            tc.tile_pool(name="wops", bufs=2, space="PSUM") as wops,
            tc.tile_pool(name="dramp", bufs=2, space="DRAM") as dramp,
        ):
            # ---- persistent tiles; loads emitted in need order ----
            kT_sb = [[persist.tile([P, S // 2], MMDT, tag=f"kT{j}{half}",
                                   name=f"kT{j}{half}")
                      for half in range(2)] for j in range(2)]
            qT_sb = [[persist.tile([P, QW], MMDT, tag=f"qT{j}{s}",
                                   name=f"qT{j}{s}")
                      for s in range(NSTRIP)] for j in range(2)]
            # V_ext = [V | 1]: row 64 of the PV product is the softmax denom
            vext_sb = [persist.tile([P, 16, D + 1], MMDT, tag=f"vext{h}",
                                    name=f"vext{h}") for h in range(H)]
            wo_sb = [persist.tile([P, D_MODEL], MMDT, tag=f"wo{j}",
                                  name=f"wo{j}") for j in range(2)]
            tri_sb = persist.tile([P, P], MMDT, tag="tri", name="tri_sb")

            nc.sync.dma_start(kT_sb[0][0][:], kT[0:2, :, 0 : S // 2].rearrange("h d s -> (h d) s"))
            nc.sync.dma_start(qT_sb[0][0][:], qT[0:2, :, 0:QW].rearrange("h d s -> (h d) s"))
            nc.sync.dma_start(tri_sb[:], tri[:])
            for h in (0, 1):
                nc.vector.memset(vext_sb[h][:, :, D : D + 1], 1.0)
                nc.sync.dma_start(
                    vext_sb[h][:, :, 0:D],
                    v[h].rearrange("p (t d) -> p t d", d=D),
                )
            nc.sync.dma_start(qT_sb[0][1][:], qT[0:2, :, QW : 2 * QW].rearrange("h d s -> (h d) s"))
            nc.sync.dma_start(kT_sb[1][0][:], kT[2:4, :, 0 : S // 2].rearrange("h d s -> (h d) s"))
            for h in (2, 3):
                nc.vector.memset(vext_sb[h][:, :, D : D + 1], 1.0)
                nc.sync.dma_start(
                    vext_sb[h][:, :, 0:D],
                    v[h].rearrange("p (t d) -> p t d", d=D),
                )
            nc.sync.dma_start(qT_sb[1][0][:], qT[2:4, :, 0:QW].rearrange("h d s -> (h d) s"))
            nc.sync.dma_start(qT_sb[1][1][:], qT[2:4, :, QW : 2 * QW].rearrange("h d s -> (h d) s"))
            nc.sync.dma_start(kT_sb[0][1][:], kT[0:2, :, S // 2 : S].rearrange("h d s -> (h d) s"))
            nc.sync.dma_start(kT_sb[1][1][:], kT[2:4, :, S // 2 : S].rearrange("h d s -> (h d) s"))
            for j in range(2):
                nc.sync.dma_start(wo_sb[j][:], wo[P * j : P * (j + 1), :])
            for s in (2, 3):
                for j in range(2):
                    nc.sync.dma_start(qT_sb[j][s][:], qT[2 * j : 2 * j + 2, :, s * QW : (s + 1) * QW].rearrange("h d s -> (h d) s"))

            def kslice(j, off, t):
                half, col = t // 8, (t % 8) * P
                return kT_sb[j][half][off : off + D, col : col + P]

            zn_sb = {}     # (strip, j) -> zn tile
            ot_sb = {}

            def emit_proj_units(sp, units):
                """Projection for strip sp over (q-block, mt-half) units."""
                for qb, mt in units:
                    ops = wops.tile([P, 512], F32, tag="wo_ps", name="wo_ps")
                    for j2 in range(2):
                        nc.tensor.matmul(
                            ops[:],
                            zn_sb[(sp, j2)][:, qb * P : (qb + 1) * P],
                            wo_sb[j2][:, mt * 512 : (mt + 1) * 512],
                            start=(j2 == 0),
                            stop=(j2 == 1),
                        )
                    if mt == 0:
                        ot_sb[(sp, qb)] = outp.tile([P, D_MODEL], ODT, tag="ot", name="ot")
                    nc.vector.tensor_copy(ot_sb[(sp, qb)][:, mt * 512 : (mt + 1) * 512], ops[:])
                    if mt == 1:
                        nc.sync.dma_start(
                            pout[(4 * sp + qb) * P : (4 * sp + qb + 1) * P, :],
                            ot_sb[(sp, qb)][:],
                        )

            # ---- main loops ----
            for s in range(NSTRIP):
                for j in range(2):
                    zn_sb[(s, j)] = znp.tile([P, QW], MMDT, tag=f"zn{j}", name=f"zn{j}")

                for h in range(H):
                    j, off = h // 2, (h % 2) * D
                    z_ps = zps.tile([D + 1, QW], F32, tag="z", name="z_ps")
                    qs = qT_sb[j][s]
                    deferred_pv = None  # one-pair PE lookahead within the head

                    # pair list: 2s full pairs, then the diagonal tiles
                    pairs = [("full", 2 * p) for p in range(2 * s)]
                    if PACK_DIAG:
                        pairs += [("dA", 4 * s), ("dB", 4 * s + 2)]
                    else:
                        pairs += [("d", 4 * s + i) for i in range(4)]

                    npairs = len(pairs)
                    for pi, (kind, t0) in enumerate(pairs):
                        sc = scps.tile([P, 2, QW], F32, tag="sc", name="sc")
                        ex = expp.tile([P, 2, QW], MMDT, tag="ex", name="ex")
                        if kind == "full":
                            for o in (0, 1):
                                nc.tensor.matmul(
                                    sc[:, o, :], kslice(j, off, t0 + o),
                                    qs[off : off + D, :], start=True, stop=True,
                                )
                            nc.scalar.activation(
                                ex[:], sc[:],
                                mybir.ActivationFunctionType.Exp, scale=0.125,
                            )
                            pv = [(t0, ex[:, 0, :], 0), (t0 + 1, ex[:, 1, :], 0)]
                        elif kind == "dA":
                            # d0: q cols [0:512); d1: q cols [128:512) packed
                            # into the second bank at offset 0
                            nc.tensor.matmul(
                                sc[:, 0, :], kslice(j, off, t0),
                                qs[off : off + D, :], start=True, stop=True,
                            )
                            nc.tensor.matmul(
                                sc[:, 1, 0:384], kslice(j, off, t0 + 1),
                                qs[off : off + D, P:QW], start=True, stop=True,
                            )
                            nc.scalar.activation(
                                ex[:, 0, :], sc[:, 0, :],
                                mybir.ActivationFunctionType.Exp, scale=0.125,
                            )
                            nc.scalar.activation(
                                ex[:, 1, 0:384], sc[:, 1, 0:384],
                                mybir.ActivationFunctionType.Exp, scale=0.125,
                            )
                            MASKENG.tensor_mul(ex[:, 0, 0:P], ex[:, 0, 0:P], tri_sb[:])
                            MASKENG.tensor_mul(ex[:, 1, 0:P], ex[:, 1, 0:P], tri_sb[:])
                            pv = [(t0, ex[:, 0, :], 0), (t0 + 1, ex[:, 1, 0:384], P)]
                        elif kind == "dB":
                            # d2: q cols [256:512); d3: q cols [384:512) packed
                            # behind it in the same bank
                            nc.tensor.matmul(
                                sc[:, 0, 0:256], kslice(j, off, t0),
                                qs[off : off + D, 2 * P : QW], start=True, stop=True,
                            )
                            nc.tensor.matmul(
                                sc[:, 0, 256:384], kslice(j, off, t0 + 1),
                                qs[off : off + D, 3 * P : QW], start=True, stop=True,
                            )
                            nc.scalar.activation(
                                ex[:, 0, 0:384], sc[:, 0, 0:384],
                                mybir.ActivationFunctionType.Exp, scale=0.125,
                            )
                            MASKENG.tensor_mul(ex[:, 0, 0:P], ex[:, 0, 0:P], tri_sb[:])
                            MASKENG.tensor_mul(ex[:, 0, 256:384], ex[:, 0, 256:384], tri_sb[:])
                            pv = [(t0, ex[:, 0, 0:256], 2 * P), (t0 + 1, ex[:, 0, 256:384], 3 * P)]
                        else:  # single diagonal tile (baseline style)
                            li = (t0 - 4 * s) * P
                            nc.tensor.matmul(
                                sc[:, 0, li:QW], kslice(j, off, t0),
                                qs[off : off + D, li:QW], start=True, stop=True,
                            )
                            nc.scalar.activation(
                                ex[:, 0, li:QW], sc[:, 0, li:QW],
                                mybir.ActivationFunctionType.Exp, scale=0.125,
                            )
                            MASKENG.tensor_mul(ex[:, 0, li : li + P], ex[:, 0, li : li + P], tri_sb[:])
                            pv = [(t0, ex[:, 0, li:QW], li)]

                        if deferred_pv is not None:
                            for tt, rhs, qoff, first, last in deferred_pv:
                                nc.tensor.matmul(
                                    z_ps[:, qoff:QW] if qoff else z_ps[:],
                                    vext_sb[h][:, tt, :], rhs,
                                    start=first, stop=last,
                                )
                        is_last_pair = pi == npairs - 1
                        deferred_pv = [
                            (e[0], e[1], e[2], e[0] == 0,
                             is_last_pair and (i == len(pv) - 1))
                            for i, e in enumerate(pv)
                        ]

                    # flush the last pair's PV, then normalize
                    for tt, rhs, qoff, first, last in deferred_pv:
                        nc.tensor.matmul(
                            z_ps[:, qoff:QW] if qoff else z_ps[:],
                            vext_sb[h][:, tt, :], rhs,
                            start=first, stop=last,
                        )

                    # normalize: zn = z[0:64] * (1 / z[64]); approx recip is
                    # exact to ~4e-6, far below bf16 input rounding
                    dcp = rcpp.tile([1, QW], F32, tag="dcp", name="dcp")
                    nc.vector.tensor_copy(dcp[:], z_ps[D : D + 1, :])
                    rcp = rcpp.tile([1, QW], F32, tag="rcp", name="rcp")
                    nc.vector.reciprocal_approx_fast(rcp[:], dcp[:])
                    rb_sb = rcpp.tile([D, QW], F32, tag="rb_sb", name="rb_sb")
                    if USE_GPS_BCAST:
                        nc.gpsimd.partition_broadcast(rb_sb[:], rcp[:], channels=D)
                    else:
                        # broadcast 1/denom across the 64 d-partitions via a
                        # DRAM bounce: DRAM sources allow a step-0 partition dim
                        rdr = dramp.tile([1, QW], F32, tag="rdr", name="rdr")
                        nc.sync.dma_start(rdr[:], rcp[:])
                        nc.sync.dma_start(
                            rb_sb[:],
                            bass.AP(tensor=rdr.tensor, offset=rdr.offset,
                                    ap=[[0, D]] + [list(a) for a in rdr.ap][1:]),
                        )
                    nc.vector.tensor_mul(
                        zn_sb[(s, j)][off : off + D, :], z_ps[0:D, :], rb_sb[:]
                    )

                    # PE filler at head boundaries: previous strip's projection
                    if s > 0:
                        if h == 1:
                            emit_proj_units(s - 1, [(0, 0), (0, 1), (1, 0), (1, 1)])
                        elif h == 2:
                            emit_proj_units(s - 1, [(2, 0), (2, 1), (3, 0), (3, 1)])

            # tail: last strip's projection
            emit_proj_units(NSTRIP - 1, [(qb, mt) for qb in range(4) for mt in range(2)])

    nc.finalize()
    return nc


def _get_program():
    global _PROGRAM, _PROGRAM_KEY
    key = (PACK_DIAG, USE_GPS_MASK, USE_GPS_BCAST, USE_BF16_OUT)
    if _PROGRAM is None or _PROGRAM_KEY != key:
        _PROGRAM = build_program()
        _PROGRAM_KEY = key
    return _PROGRAM


def make_in_maps(q, k, v, W_O, n_cores=N_CORES):
    """Shard full inputs into per-core maps (core = batch*4 + head_group)."""
    import ml_dtypes
    mmdt = ml_dtypes.bfloat16
    q = np.ascontiguousarray(np.asarray(q, dtype=np.float32))
    k = np.ascontiguousarray(np.asarray(k, dtype=np.float32))
    v = np.ascontiguousarray(np.asarray(v, dtype=np.float32))
    W_O = np.ascontiguousarray(np.asarray(W_O, dtype=np.float32))
    B = q.shape[0]
    qT = np.ascontiguousarray(q.reshape(B, S, N_HEADS, D).transpose(0, 2, 3, 1))
    kT = np.ascontiguousarray(k.reshape(B, S, N_HEADS, D).transpose(0, 2, 3, 1))
    # v: [b, h, S, d] -> [b, h, p(kv%128), (kvtile(16) d)]
    vh = v.reshape(B, S, N_HEADS, D).transpose(0, 2, 1, 3)
    vh = vh.reshape(B, N_HEADS, 16, P, D).transpose(0, 1, 3, 2, 4)
    vh = np.ascontiguousarray(vh.reshape(B, N_HEADS, P, 16 * D))
    # mask[kv, q] = 1 iff kv <= q  (scores live transposed: partition=kv, free=q)
    tri = np.ascontiguousarray(np.triu(np.ones((P, P), dtype=np.float32)))
    in_maps = []
    for core in range(n_cores):
        b, g = core // 4, core % 4
        hs = slice(H * g, H * (g + 1))
        in_maps.append(
            {
                "qT": np.ascontiguousarray(qT[b, hs]).astype(mmdt),
                "kT": np.ascontiguousarray(kT[b, hs]).astype(mmdt),
                "v": np.ascontiguousarray(vh[b, hs]).astype(mmdt),
                "wo": np.ascontiguousarray(W_O[hs].reshape(2 * P, D_MODEL)).astype(mmdt),
                "tri": tri.astype(mmdt),
            }
        )
    return in_maps


def kernel(residual, q, k, v, W_O, b_O, _trace=False, _trace_kwargs=None):
    global LAST_RESULTS
    residual = np.asarray(residual, dtype=np.float32)
    B = residual.shape[0]
    in_maps = make_in_maps(q, k, v, W_O)
    nc = _get_program()
    res = run_bass_kernel_spmd(
        nc, in_maps, list(range(N_CORES)), trace=_trace, **(_trace_kwargs or {})
    )
    LAST_RESULTS = res
    out = np.zeros((B, S, D_MODEL), dtype=np.float64)
    for core in range(N_CORES):
        out[core // 4] += res.results[core]["pout"].astype(np.float64)
    out += np.asarray(b_O, dtype=np.float64)
    return (residual, out.astype(np.float32))
